# revision 1
# baseline (speedup 1.0000x reference)
"""BiLSTM-CRF on 8 Trainium2 NeuronCores (Bass/Tile), data-parallel over batch.

Each core owns 16 batch rows; tokens are laid out (s, b). The device computes
the char convs (one-hot matmuls against host-precomputed char_emb @ W tables),
the BiLSTM (all matmul offsets static via block staging), the emissions
projection, and the full CRF (gold score + logZ with colsum renormalisation).
The host does the word-embedding gather, packing, and the final reduction.

A tiny warmup NEFF is dispatched on a background thread at entry: the first
run_bass_kernel_spmd call in a process pays a large one-time init cost that is
graph-size dependent, so we pay it on a 5-instruction kernel concurrently with
host prep + graph build.

Falls back to a pure-numpy host path on any device failure.
"""

import os
import sys
import threading

sys.path.insert(0, "/opt/trn_rl_repo")

import numpy as np

try:
    import ml_dtypes

    BF16 = ml_dtypes.bfloat16
except Exception:  # pragma: no cover
    BF16 = None

B, S, C = 128, 512, 16
V, CV = 50000, 100
E, CE = 100, 30
H = 128
T = 17
NF = 25
LSTM_IN = E + 3 * NF  # 175
N_CORES = 8
B_LOC = B // N_CORES  # 16
N_TOK = B_LOC * S  # 8192
NPOS = N_TOK * C  # 131072
C_SHIFT = float(np.log(T))
N_SHIFTS = 9
SHIFT_K = [2, 2, 3, 3, 3, 4, 4, 4, 4]
SHIFT_OF = [0, 1, 0, 1, 2, 0, 1, 2, 3]
SHIFT_BLK = [0, 0, 1, 1, 1, 2, 2, 2, 2]

LAST_EXEC_NS = None

_STATE = {}


# ---------------------------------------------------------------------------
# BIR post-processing: this walrus build accepts at most ONE sync wait per
# instruction.  Hoist excess waits onto NoOps inserted just before (engine
# execution is in-order, so this is semantics-preserving).
# ---------------------------------------------------------------------------

def _split_waits_json(bir_json: bytes) -> bytes:
    import orjson

    m = orjson.loads(bir_json)
    ctr = 0
    for fn in m["functions"]:
        for b in fn.get("blocks") or []:
            instructions = b.get("instructions")
            if not instructions:
                continue
            out = []
            for ins in instructions:
                si = ins.get("sync_info")
                waits = (si or {}).get("on_wait") or []
                if len(waits) > 1:
                    for w in waits[:-1]:
                        ctr += 1
                        out.append({
                            "engine": ins["engine"],
                            "ins": [],
                            "outs": [],
                            "name": f"{ins['name']}_wsplit{ctr}",
                            "opcode": "NoOp",
                            "sync_info": {"on_update": [], "on_wait": [w]},
                        })
                    si["on_wait"] = waits[-1:]
                out.append(ins)
            b["instructions"] = out
    return orjson.dumps(m)


def _patch_compiler():
    if _STATE.get("patched"):
        return
    from concourse import bass2jax, bass_utils

    orig = bass_utils.compile_bir_kernel

    def patched(bir_json, tmpdir, neff_name="file.neff"):
        return orig(_split_waits_json(bir_json), tmpdir, neff_name)

    bass2jax.compile_bir_kernel = patched
    _STATE["patched"] = True


def _build_warmup_nc():
    import concourse.bass as bass
    import concourse.mybir as mybir
    from concourse import tile

    nc = bass.Bass()
    x_d = nc.declare_dram_parameter("x", [1, 8], mybir.dt.float32, isOutput=False)
    y_d = nc.declare_dram_parameter("y", [1, 8], mybir.dt.float32, isOutput=True)
    with tile.TileContext(nc) as tc:
        with tc.tile_pool(name="s", bufs=1) as sp:
            t = sp.tile([1, 8], mybir.dt.float32, tag="t")
            nc.sync.dma_start(t[:], x_d[:])
            nc.scalar.mul(t[:], t[:], 2.0)
            nc.sync.dma_start(y_d[:], t[:])
    return nc


def _warmup(nc):
    """Dispatch a trivial NEFF to absorb the per-process first-call cost.

    The graph is built by the caller on the main thread; this thread only
    lowers/dispatches (no concurrent bass graph builds)."""
    from concourse.bass_utils import run_bass_kernel_spmd

    run_bass_kernel_spmd(
        nc, [{"x": np.ones((1, 8), np.float32)}] * N_CORES,
        core_ids=list(range(N_CORES)))


# ---------------------------------------------------------------------------
# Device kernel
# ---------------------------------------------------------------------------

def _build_nc():
    import concourse.bass as bass
    import concourse.mybir as mybir
    from concourse import tile

    f32 = mybir.dt.float32
    bf16 = mybir.dt.bfloat16
    AF = mybir.ActivationFunctionType
    OP = mybir.AluOpType
    AX = mybir.AxisListType
    ds = bass.ds

    nc = bass.Bass()

    dp = lambda n, shp, dt: nc.declare_dram_parameter(n, shp, dt, isOutput=False)
    # wepack rows: 0 tags, 1..100 word embeddings (all bf16, one DMA)
    wepack_d = dp("wepack", [E + 1, N_TOK], bf16)
    cids_d = dp("cids", [1, NPOS], bf16)
    # wpack cols: [wih0f|wih0b|whhf|whhb|wih1f|wih1b] (512 each) |convt 288|h2tf 17|h2tb 17
    wpack_d = dp("wpack", [128, 6 * 512 + N_SHIFTS * 32 + 2 * T], bf16)
    # cpack cols: 0 convb, 1 startv, 2 endv, 3 h2tbias, 5.. expT, 22.. trans (f32)
    cpack_d = dp("cpack", [96, 4 + 1 + 2 * T], f32)
    out_d = nc.declare_dram_parameter("out", [1, 8], f32, isOutput=True)
    chk_d = nc.declare_dram_parameter("chk", [128, 4], f32, isOutput=True)

    with tile.TileContext(nc) as tc:
        with (
            tc.tile_pool(name="persist", bufs=1) as pers,
            tc.tile_pool(name="work", bufs=3) as work,
            tc.tile_pool(name="ids", bufs=3) as idsp,
        ):
            feats0 = pers.tile([128, N_TOK], bf16, tag="feats0")
            feats1 = pers.tile([69, N_TOK], bf16, tag="feats1")
            h_f = pers.tile([H, N_TOK + B_LOC], bf16, tag="h_f")
            h_b = pers.tile([H, N_TOK + B_LOC], bf16, tag="h_b")
            em = pers.tile([T, N_TOK], f32, tag="em")
            ohb = pers.tile([T, N_TOK], f32, tag="ohb")
            iota_i = pers.tile([128, 512], mybir.dt.int32, tag="iota_i")
            iota_f = pers.tile([128, 512], f32, tag="iota_f")
            wpack = pers.tile([128, 6 * 512 + N_SHIFTS * 32 + 2 * T], bf16, tag="wpack")
            cpack = pers.tile([96, 4 + 1 + 2 * T], f32, tag="cpack")
            wetags = pers.tile([E + 1, N_TOK], bf16, tag="wetags")
            wih0f = wpack[:, 0:512]
            wih0b = wpack[:, 512:1024]
            whhf = wpack[:, 1024:1536]
            whhb = wpack[:, 1536:2048]
            wih1f = wpack[0:69, 2048:2560]
            wih1b = wpack[0:69, 2560:3072]
            convt = wpack[0:E, 3072:3072 + N_SHIFTS * 32]
            h2tf = wpack[:, 3360:3360 + T]
            h2tb = wpack[:, 3360 + T:3360 + 2 * T]
            convb = cpack[:, 0:1]
            startv = cpack[0:T, 1:2]
            endv = cpack[0:T, 2:3]
            h2tbias = cpack[0:T, 3:4]
            expT = cpack[0:T, 5:5 + T]
            transm = cpack[0:T, 5 + T:5 + 2 * T]
            tagsb = wetags[0:1, :]
            ones1b = pers.tile([1, 128], bf16, tag="ones1b")
            ones17f = pers.tile([T, 1], f32, tag="ones17f")
            ones117f = pers.tile([1, T], f32, tag="ones117f")
            ones16f = pers.tile([T, 16], f32, tag="ones16f")
            alpha = pers.tile([T, B_LOC], f32, tag="alpha")
            off = pers.tile([1, B_LOC], f32, tag="off")
            cst_f = pers.tile([128, B_LOC], f32, tag="cst_f")
            cst_b = pers.tile([128, B_LOC], f32, tag="cst_b")
            acc_em = pers.tile([T, 16], f32, tag="acc_em")
            acc_tr = pers.tile([T, 16], f32, tag="acc_tr")
            acc_se = pers.tile([T, 2], f32, tag="acc_se")
            out_sb = pers.tile([1, 8], f32, tag="out_sb")
            junk = pers.tile([T, 512], f32, tag="junk")
            junk2 = pers.tile([T, 512], f32, tag="junk2")
            junk3 = pers.tile([T, 16], f32, tag="junk3")
            junk4 = pers.tile([T, 16], f32, tag="junk4")
            lzrow = pers.tile([1, B_LOC], f32, tag="lzrow")
            negC = pers.tile([T, 1], f32, tag="negC")

            nc.sync.dma_start(wpack[:], wpack_d[:])
            nc.sync.dma_start(cpack[:], cpack_d[:])
            nc.sync.dma_start(wetags[:], wepack_d[:])
            nc.sync.dma_start(feats0[96:128, :], wepack_d[1:33, :])
            nc.sync.dma_start(feats1[1:69, :], wepack_d[33:E + 1, :])
            # input checksums: per-partition sums of the packed loads, written
            # out for host-side staleness detection
            chk_sb = pers.tile([128, 4], f32, tag="chk_sb")
            nc.vector.memset(chk_sb[:], 0.0)
            nc.vector.tensor_reduce(chk_sb[:, 0:1], wpack[:], axis=AX.X, op=OP.add)
            nc.vector.tensor_reduce(chk_sb[0:96, 1:2], cpack[:], axis=AX.X, op=OP.add)
            nc.vector.tensor_reduce(chk_sb[0:E + 1, 2:3], wetags[:], axis=AX.X, op=OP.add)
            nc.sync.dma_start(chk_d[:], chk_sb[:])

            nc.gpsimd.iota(iota_i[:], pattern=[[0, 512]], base=0, channel_multiplier=1)
            nc.vector.tensor_copy(iota_f[:], iota_i[:])
            nc.vector.memset(ones1b[:], 1.0)
            nc.vector.memset(ones17f[:], 1.0)
            nc.vector.memset(ones117f[:], 1.0)
            nc.vector.memset(ones16f[:], 1.0)
            nc.vector.memset(feats1[0:1, :], 1.0)
            nc.vector.memset(h_f[:, 0:B_LOC], 0.0)
            nc.vector.memset(h_b[:, N_TOK:N_TOK + B_LOC], 0.0)
            nc.vector.memset(cst_f[:], 0.0)
            nc.vector.memset(cst_b[:], 0.0)
            nc.vector.memset(off[:], 0.0)
            nc.vector.memset(out_sb[:], 0.0)
            nc.vector.memset(negC[:], -C_SHIFT)
            mask3 = pers.tile([96, 512], f32, tag="mask3")
            nc.vector.memset(mask3[:], 1.0)
            m3 = mask3[:].rearrange("p (t c) -> p t c", c=16)
            for blk, k in enumerate((2, 3, 4)):
                nc.gpsimd.affine_select(
                    m3[blk * 32:blk * 32 + NF], m3[blk * 32:blk * 32 + NF],
                    pattern=[[0, 32], [-1, 16]], compare_op=OP.is_ge,
                    fill=0.0, base=16 - k, channel_multiplier=0,
                )

            # ---- phase 1: char convs ----
            CHAR_UNROLL = 4
            with tc.tile_pool(name="ps_char", bufs=2, space="PSUM") as pchar:
                with tc.For_i(0, NPOS // 512 // CHAR_UNROLL) as jo:
                    idst = idsp.tile([1, 512 * CHAR_UNROLL], bf16, tag="idst")
                    nc.sync.dma_start(idst[:], cids_d[0:1, ds(jo * (512 * CHAR_UNROLL), 512 * CHAR_UNROLL)])
                    for ji in range(CHAR_UNROLL):
                        ps_ids = pchar.tile([128, 512], f32, tag="ps_ids")
                        nc.tensor.matmul(ps_ids[:], ones1b[:, 0:128], idst[0:1, ji * 512:(ji + 1) * 512],
                                         start=True, stop=True)
                        oh = work.tile([128, 516], bf16, tag="oh_c")
                        nc.vector.tensor_tensor(oh[:, 0:512], ps_ids[:], iota_f[:], op=OP.is_equal)
                        nc.vector.memset(oh[:, 512:516], 0.0)
                        ps_cv = pchar.tile([96, 512], f32, tag="ps_cv")
                        for si in range(N_SHIFTS):
                            blk = SHIFT_BLK[si]
                            nc.tensor.matmul(
                                ps_cv[blk * 32:(blk + 1) * 32, :],
                                convt[:, si * 32:(si + 1) * 32],
                                oh[0:E, SHIFT_OF[si]:SHIFT_OF[si] + 512],
                                start=si in (0, 2, 5), stop=si in (1, 4, 8),
                            )
                        cv = work.tile([96, 512], f32, tag="cv")
                        nc.scalar.activation(cv[:], ps_cv[:], AF.Relu, bias=convb)
                        nc.vector.tensor_tensor(cv[:], cv[:], mask3[:], op=OP.mult)
                        cv3 = cv[:].rearrange("p (t c) -> p t c", c=16)
                        nc.vector.tensor_reduce(
                            feats0[0:96, ds(jo * (32 * CHAR_UNROLL) + ji * 32, 32)],
                            cv3, axis=AX.X, op=OP.max)

            # ---- phase 2: BiLSTM (block-staged; all matmul offsets static) ----
            LSTM_BLK = 8
            BW = 16 * LSTM_BLK
            hs_f = pers.tile([H, BW + 16], bf16, tag="hs_f")
            hs_b = pers.tile([H, BW + 16], bf16, tag="hs_b")
            # iteration 0's carry copy reads these: they are the true h0/c0 zeros
            nc.vector.memset(hs_f[:, BW:BW + 16], 0.0)
            nc.vector.memset(hs_b[:, 0:16], 0.0)
            with tc.tile_pool(name="ps_lstm", bufs=2, space="PSUM") as plstm:
                with tc.For_i(0, S // LSTM_BLK) as ko:
                    nc.vector.tensor_copy(hs_f[:, 0:16], hs_f[:, BW:BW + 16])
                    nc.vector.tensor_copy(hs_b[:, BW:BW + 16], hs_b[:, 0:16])
                    f0f = work.tile([128, BW], bf16, tag="f0f")
                    f1f = work.tile([69, BW], bf16, tag="f1f")
                    f0b = work.tile([128, BW], bf16, tag="f0b")
                    f1b = work.tile([69, BW], bf16, tag="f1b")
                    nc.vector.tensor_copy(f0f[:], feats0[:, ds(ko * BW, BW)])
                    nc.gpsimd.tensor_copy(f1f[:], feats1[:, ds(ko * BW, BW)])
                    nc.vector.tensor_copy(f0b[:], feats0[:, ds(N_TOK - BW - ko * BW, BW)])
                    nc.gpsimd.tensor_copy(f1b[:], feats1[:, ds(N_TOK - BW - ko * BW, BW)])
                    for j in range(LSTM_BLK):
                        for is_f in (True, False):
                            if is_f:
                                w0, w1, wh, cst = wih0f, wih1f, whhf, cst_f
                                fc = slice(j * 16, (j + 1) * 16)
                                hr = slice(j * 16, (j + 1) * 16)
                                hw = slice((j + 1) * 16, (j + 2) * 16)
                                f0s, f1s, hs = f0f, f1f, hs_f
                            else:
                                w0, w1, wh, cst = wih0b, wih1b, whhb, cst_b
                                fc = slice((LSTM_BLK - 1 - j) * 16, (LSTM_BLK - j) * 16)
                                hr = slice((LSTM_BLK - j) * 16, (LSTM_BLK - j + 1) * 16)
                                hw = slice((LSTM_BLK - 1 - j) * 16, (LSTM_BLK - j) * 16)
                                f0s, f1s, hs = f0b, f1b, hs_b
                            ps_g = plstm.tile([128, 64], f32, tag="ps_gf" if is_f else "ps_gb")
                            for g in range(4):
                                gs = slice(g * 128, (g + 1) * 128)
                                gc = slice(g * 16, (g + 1) * 16)
                                nc.tensor.matmul(ps_g[:, gc], w0[:, gs], f0s[:, fc], start=True, stop=False)
                                nc.tensor.matmul(ps_g[:, gc], w1[:, gs], f1s[:, fc], start=False, stop=False)
                                nc.tensor.matmul(ps_g[:, gc], wh[:, gs], hs[:, hr], start=False, stop=True)
                            sig = work.tile([128, 48], f32, tag="sig_f" if is_f else "sig_b")
                            gg = work.tile([128, 16], f32, tag="gg_f" if is_f else "gg_b")
                            nc.scalar.activation(sig[:], ps_g[:, 0:48], AF.Sigmoid)
                            nc.scalar.activation(gg[:], ps_g[:, 48:64], AF.Tanh)
                            tmp = work.tile([128, 16], f32, tag="tmp_f" if is_f else "tmp_b")
                            nc.vector.tensor_tensor(tmp[:], sig[:, 0:16], gg[:], op=OP.mult)
                            nc.vector.tensor_tensor(cst[:], cst[:], sig[:, 16:32], op=OP.mult)
                            nc.vector.tensor_tensor(cst[:], cst[:], tmp[:], op=OP.add)
                            th = work.tile([128, 16], f32, tag="th_f" if is_f else "th_b")
                            nc.scalar.activation(th[:], cst[:], AF.Tanh)
                            nc.vector.tensor_tensor(hs[:, hw], sig[:, 32:48], th[:], op=OP.mult)
                    nc.gpsimd.tensor_copy(h_f[:, ds(ko * BW + 16, BW)], hs_f[:, 16:BW + 16])
                    nc.gpsimd.tensor_copy(h_b[:, ds(N_TOK - BW - ko * BW, BW)], hs_b[:, 0:BW])

            # ---- phase 3: emissions ----
            with tc.tile_pool(name="ps_em", bufs=2, space="PSUM") as pem:
                for j in range(N_TOK // 512):
                    ps_e = pem.tile([T, 512], f32, tag="ps_e")
                    nc.tensor.matmul(ps_e[:], h2tf, h_f[:, j * 512 + 16:(j + 1) * 512 + 16], start=True, stop=False)
                    nc.tensor.matmul(ps_e[:], h2tb, h_b[:, j * 512:(j + 1) * 512], start=False, stop=True)
                    nc.scalar.activation(em[:, j * 512:(j + 1) * 512], ps_e[:], AF.Identity, bias=h2tbias)

            # ---- phase 4: tag one-hots + gold score ----
            with tc.tile_pool(name="ps_sc", bufs=2, space="PSUM") as psc:
                for j in range(16):
                    ps_tg = psc.tile([T, 512], f32, tag="ps_tg")
                    nc.tensor.matmul(ps_tg[:], ones1b[:, 0:T], tagsb[:, j * 512:(j + 1) * 512], start=True, stop=True)
                    nc.vector.tensor_tensor(ohb[:, j * 512:(j + 1) * 512], ps_tg[:], iota_f[0:T, :], op=OP.is_equal)
                for j in range(16):
                    nc.vector.scalar_tensor_tensor(
                        junk[:], em[:, j * 512:(j + 1) * 512], 1.0, ohb[:, j * 512:(j + 1) * 512],
                        op0=OP.mult, op1=OP.mult, accum_out=acc_em[:, j:j + 1])
                for j in range(16):
                    ps_t2 = psc.tile([T, 512], f32, tag="ps_t2")
                    nc.tensor.matmul(ps_t2[:], transm, ohb[:, j * 512:(j + 1) * 512], start=True, stop=True)
                    w = 512 if j < 15 else 496
                    nc.vector.scalar_tensor_tensor(
                        junk2[:, 0:w], ps_t2[:, 0:w], 1.0, ohb[:, j * 512 + 16:j * 512 + 16 + w],
                        op0=OP.mult, op1=OP.mult, accum_out=acc_tr[:, j:j + 1])
                nc.vector.scalar_tensor_tensor(
                    junk3[:], ohb[:, 0:16], startv, ones16f[:],
                    op0=OP.mult, op1=OP.mult, accum_out=acc_se[:, 0:1])
                nc.vector.scalar_tensor_tensor(
                    junk4[:], ohb[:, N_TOK - 16:N_TOK], endv, ones16f[:],
                    op0=OP.mult, op1=OP.mult, accum_out=acc_se[:, 1:2])
                r1 = work.tile([T, 1], f32, tag="r1")
                nc.vector.tensor_reduce(r1[:], acc_em[:], axis=AX.X, op=OP.add)
                r2 = work.tile([T, 1], f32, tag="r2")
                nc.vector.tensor_reduce(r2[:], acc_tr[:], axis=AX.X, op=OP.add)
                r3 = work.tile([T, 1], f32, tag="r3")
                nc.vector.tensor_reduce(r3[:], acc_se[:], axis=AX.X, op=OP.add)
                nc.vector.tensor_tensor(r1[:], r1[:], r2[:], op=OP.add)
                nc.vector.tensor_tensor(r1[:], r1[:], r3[:], op=OP.add)
                ps_sc1 = psc.tile([1, 1], f32, tag="ps_sc1")
                nc.tensor.matmul(ps_sc1[:], ones17f[:], r1[:], start=True, stop=True)
                nc.vector.tensor_copy(out_sb[0:1, 0:1], ps_sc1[:])

            # ---- phase 5: CRF logZ forward scan ----
            with tc.tile_pool(name="ps_crf", bufs=2, space="PSUM") as pcrf:
                nc.scalar.activation(alpha[:], em[:, 0:B_LOC], AF.Identity, bias=startv)

                def crf_step(src_tile, em_slice):
                    exps = work.tile([T, B_LOC], f32, tag="exps")
                    nc.scalar.activation(exps[:], alpha[:], AF.Exp, bias=negC[:])
                    ps_a = pcrf.tile([T, B_LOC], f32, tag="ps_a")
                    nc.tensor.matmul(ps_a[:], expT, exps[:], start=True, stop=True)
                    lna = work.tile([T, B_LOC], f32, tag="lna")
                    nc.scalar.activation(lna[:], ps_a[:], AF.Ln)
                    nc.vector.tensor_tensor(alpha[:], lna[:], src_tile[:, em_slice], op=OP.add)

                def renorm():
                    exps = work.tile([T, B_LOC], f32, tag="exps")
                    nc.scalar.activation(exps[:], alpha[:], AF.Exp)
                    ps_s = pcrf.tile([1, B_LOC], f32, tag="ps_s")
                    nc.tensor.matmul(ps_s[:], ones17f[:], exps[:], start=True, stop=True)
                    lns = work.tile([1, B_LOC], f32, tag="lns")
                    nc.scalar.activation(lns[:], ps_s[:], AF.Ln)
                    nc.vector.tensor_tensor(off[:], off[:], lns[:], op=OP.add)
                    ps_m = pcrf.tile([T, B_LOC], f32, tag="ps_m")
                    nc.tensor.matmul(ps_m[:], ones117f[:], lns[:], start=True, stop=True)
                    nc.vector.tensor_tensor(alpha[:], alpha[:], ps_m[:], op=OP.subtract)

                with tc.For_i(0, 31) as ko:
                    em_stage = work.tile([T, 256], f32, tag="em_stage")
                    nc.vector.tensor_copy(em_stage[:], em[:, ds(ko * 256 + 16, 256)])
                    for kj in range(16):
                        crf_step(em_stage, slice(kj * 16, (kj + 1) * 16))
                    renorm()
                for t in range(497, 512):
                    crf_step(em, slice(t * 16, (t + 1) * 16))

                exps2 = work.tile([T, B_LOC], f32, tag="exps2")
                nc.scalar.activation(exps2[:], alpha[:], AF.Exp, bias=endv)
                ps_s2 = pcrf.tile([1, B_LOC], f32, tag="ps_s2")
                nc.tensor.matmul(ps_s2[:], ones17f[:], exps2[:], start=True, stop=True)
                lns2 = work.tile([1, B_LOC], f32, tag="lns2")
                nc.scalar.activation(lns2[:], ps_s2[:], AF.Ln)
                nc.vector.tensor_tensor(lzrow[:], lns2[:], off[:], op=OP.add)
                nc.vector.tensor_reduce(out_sb[0:1, 1:2], lzrow[:], axis=AX.X, op=OP.add)

            nc.sync.dma_start(out_d[:], out_sb[:])

    return nc


def _host_prep(inputs):
    xi = np.asarray(inputs["x"]).astype(np.int64)
    cxi = np.asarray(inputs["char_x"]).astype(np.int64)
    tg = np.asarray(inputs["tags"]).astype(np.int64)
    word_emb = np.asarray(inputs["word_emb"], np.float32)
    char_emb = np.asarray(inputs["char_emb"], np.float32)

    def gate_perm(w):
        return np.concatenate([w[0:H], w[H:2 * H], w[3 * H:4 * H], w[2 * H:3 * H]], axis=0)

    def prep_lstm(W_ih, W_hh, b):
        W_ih = gate_perm(np.asarray(W_ih, np.float32))
        W_hh = gate_perm(np.asarray(W_hh, np.float32))
        b = gate_perm(np.asarray(b, np.float32).reshape(4 * H, 1))[:, 0]
        wihT = W_ih.T  # [175, 512]; cols 0..99 we, 100..174 conv feats
        wih0 = np.zeros((128, 4 * H), np.float32)
        for blk in range(3):
            wih0[blk * 32:blk * 32 + NF] = wihT[E + blk * NF:E + (blk + 1) * NF]
        wih0[96:128] = wihT[0:32]
        wih1 = np.zeros((128, 4 * H), np.float32)
        wih1[0] = b
        wih1[1:69] = wihT[32:E]
        whh = np.zeros((128, 4 * H), np.float32)
        whh[:] = W_hh.T
        return wih0, wih1, whh

    wih0f, wih1f, whhf = prep_lstm(inputs["W_ih_f"], inputs["W_hh_f"], inputs["b_f"])
    wih0b, wih1b, whhb = prep_lstm(inputs["W_ih_b"], inputs["W_hh_b"], inputs["b_b"])

    convt = np.zeros((128, N_SHIFTS * 32), np.float32)
    for si, (k, sh) in enumerate(zip(SHIFT_K, SHIFT_OF)):
        Wk = np.asarray(inputs[f"conv{k}_W"], np.float32)
        convt[0:E, si * 32:si * 32 + NF] = char_emb @ Wk[:, :, sh].T

    h2t_W = np.asarray(inputs["h2t_W"], np.float32)
    h2t = np.zeros((128, 2 * T), np.float32)
    h2t[:, 0:T] = h2t_W[:, 0:H].T
    h2t[:, T:2 * T] = h2t_W[:, H:2 * H].T

    wpack = np.concatenate(
        [wih0f, wih0b, whhf, whhb, wih1f, wih1b, convt, h2t], axis=1
    ).astype(BF16)

    cpack = np.zeros((96, 4 + 1 + 2 * T), np.float32)
    cpack[0:NF, 0] = np.asarray(inputs["conv2_b"], np.float32)
    cpack[32:32 + NF, 0] = np.asarray(inputs["conv3_b"], np.float32)
    cpack[64:64 + NF, 0] = np.asarray(inputs["conv4_b"], np.float32)
    cpack[0:T, 1] = np.asarray(inputs["crf_start"], np.float32)
    cpack[0:T, 2] = np.asarray(inputs["crf_end"], np.float32)
    cpack[0:T, 3] = np.asarray(inputs["h2t_b"], np.float32)
    trans = np.asarray(inputs["crf_trans"], np.float32)
    cpack[0:T, 5:5 + T] = np.exp(trans)
    cpack[0:T, 5 + T:5 + 2 * T] = trans

    we_all = word_emb[xi].astype(BF16)  # [B, S, E]

    in_maps = []
    expect_chk = []
    for c in range(N_CORES):
        rows = slice(c * B_LOC, (c + 1) * B_LOC)
        wepack = np.zeros((E + 1, N_TOK), BF16)
        wepack[0] = tg[rows].T.reshape(N_TOK).astype(BF16)
        wepack[1:E + 1] = we_all[rows].transpose(2, 1, 0).reshape(E, N_TOK)
        cids_c = np.ascontiguousarray(
            cxi[rows].transpose(1, 0, 2).reshape(1, NPOS)).astype(BF16)
        in_maps.append(dict(wepack=wepack, cids=cids_c, wpack=wpack, cpack=cpack))
        expect_chk.append(
            (wpack.astype(np.float32).sum(axis=1),
             cpack.sum(axis=1),
             wepack.astype(np.float32).sum(axis=1)))
    return in_maps, expect_chk


def _run_device(inputs):
    global LAST_EXEC_NS
    import time as _time
    _patch_compiler()
    _t0 = _time.perf_counter()
    from concourse.bass_utils import run_bass_kernel_spmd

    nc_warm = _build_warmup_nc()
    warm = threading.Thread(target=_warmup, args=(nc_warm,), daemon=True)
    warm.start()
    nc = _build_nc()
    _t1 = _time.perf_counter()
    in_maps, expect_chk = _host_prep(inputs)
    _t2 = _time.perf_counter()
    warm.join(timeout=25)
    _t3 = _time.perf_counter()
    if warm.is_alive():
        # terminal is wedged or very slow; the host path (~6s) beats a
        # slow device path from here
        raise RuntimeError(f"warmup still running after {_t3 - _t2:.0f}s")

    def one_call():
        res = run_bass_kernel_spmd(nc, in_maps, core_ids=list(range(N_CORES)))
        corr = B_LOC * (S - 1) * C_SHIFT
        total = np.float64(0.0)
        ok = True
        dbg = os.environ.get("KERNEL_TIMING")
        for c in range(N_CORES):
            o = res.results[c]["out"][0]
            nll_c = (float(o[1]) + corr) - float(o[0])
            if not np.isfinite(nll_c) or not (15000.0 < nll_c < 35000.0):
                ok = False
                if dbg:
                    print(f"[implausible] core {c}: score={float(o[0]):.3f} lzrel={float(o[1]):.3f} nll={nll_c:.3f}", file=sys.stderr)
            chk = res.results[c]["chk"]
            ew, ec, ewe = expect_chk[c]
            for name, got, want, sl in (
                ("wpack", chk[:, 0], ew, slice(0, 128)),
                ("cpack", chk[0:96, 1], ec, slice(0, 96)),
                ("wepack", chk[0:E + 1, 2], ewe, slice(0, E + 1)),
            ):
                err = np.abs(got - want)
                tol = 1.0 + 0.02 * np.abs(want)
                if not (err <= tol).all():
                    ok = False
                    if dbg:
                        bad = int((err > tol).sum())
                        print(f"[stale-input] core {c} {name}: {bad} partitions mismatch", file=sys.stderr)
            total += nll_c
        return ok, np.float32(total)

    ok, total = one_call()
    if not ok:
        print("device result implausible; retrying once", file=sys.stderr)
        ok, total = one_call()
    _t4 = _time.perf_counter()
    if os.environ.get("KERNEL_TIMING"):
        print(f"[timing] build={_t1-_t0:.2f}s prep={_t2-_t1:.2f}s warmjoin={_t3-_t2:.2f}s call={_t4-_t3:.2f}s", file=sys.stderr)
    if not ok:
        raise RuntimeError("device result implausible after retry")
    return total


# ---------------------------------------------------------------------------
# Host fallback (pure numpy) — used if the device path fails.
# ---------------------------------------------------------------------------

def _sigmoid(x):
    out = np.empty_like(x)
    pos = x >= 0
    out[pos] = 1.0 / (1.0 + np.exp(-x[pos]))
    ex = np.exp(x[~pos])
    out[~pos] = ex / (1.0 + ex)
    return out


def _char_conv_np(ce, W, b):
    k = W.shape[2]
    sw = np.lib.stride_tricks.sliding_window_view(ce, k, axis=1)
    n, p = sw.shape[0], sw.shape[1]
    sw = np.ascontiguousarray(sw).reshape(n, p, CE * k)
    Wf = W.reshape(NF, CE * k).astype(np.float32)
    out = sw @ Wf.T + b[None, None, :]
    np.maximum(out, 0.0, out=out)
    return out.max(axis=1)


def _lstm_dir_np(pre, W_hh, reverse):
    n = pre.shape[0]
    h = np.zeros((n, H), np.float32)
    c = np.zeros((n, H), np.float32)
    hs = np.empty((n, S, H), np.float32)
    Wt = np.ascontiguousarray(W_hh.T)
    order = range(S - 1, -1, -1) if reverse else range(S)
    for t in order:
        g = pre[:, t] + h @ Wt
        i = _sigmoid(g[:, :H])
        f = _sigmoid(g[:, H:2 * H])
        gg = np.tanh(g[:, 2 * H:3 * H])
        o = _sigmoid(g[:, 3 * H:])
        c = f * c + i * gg
        h = o * np.tanh(c)
        hs[:, t] = h
    return hs


def _logsumexp(a, axis):
    m = a.max(axis=axis, keepdims=True)
    return (m + np.log(np.exp(a - m).sum(axis=axis, keepdims=True))).squeeze(axis)


def _run_host(inputs):
    xi = np.asarray(inputs["x"]).astype(np.int64)
    cxi = np.asarray(inputs["char_x"]).astype(np.int64)
    tg = np.asarray(inputs["tags"]).astype(np.int64)
    msk = np.asarray(inputs["mask"]).astype(bool)
    word_emb = np.asarray(inputs["word_emb"], np.float32)
    char_emb = np.asarray(inputs["char_emb"], np.float32)

    we = word_emb[xi]
    ce = char_emb[cxi].reshape(B * S, C, CE)
    cf = np.concatenate(
        [
            _char_conv_np(ce, np.asarray(inputs["conv2_W"], np.float32), np.asarray(inputs["conv2_b"], np.float32)),
            _char_conv_np(ce, np.asarray(inputs["conv3_W"], np.float32), np.asarray(inputs["conv3_b"], np.float32)),
            _char_conv_np(ce, np.asarray(inputs["conv4_W"], np.float32), np.asarray(inputs["conv4_b"], np.float32)),
        ],
        axis=1,
    ).reshape(B, S, 3 * NF)
    feats = np.concatenate([we, cf], axis=2)

    ff = feats.reshape(B * S, LSTM_IN)
    pre_f = (ff @ np.asarray(inputs["W_ih_f"], np.float32).T + np.asarray(inputs["b_f"], np.float32)).reshape(B, S, 4 * H)
    pre_b = (ff @ np.asarray(inputs["W_ih_b"], np.float32).T + np.asarray(inputs["b_b"], np.float32)).reshape(B, S, 4 * H)
    h_f = _lstm_dir_np(pre_f, np.asarray(inputs["W_hh_f"], np.float32), reverse=False)
    h_b = _lstm_dir_np(pre_b, np.asarray(inputs["W_hh_b"], np.float32), reverse=True)
    h = np.concatenate([h_f, h_b], axis=2)

    emissions = (h.reshape(B * S, 2 * H) @ np.asarray(inputs["h2t_W"], np.float32).T).reshape(B, S, T)
    emissions = emissions + np.asarray(inputs["h2t_b"], np.float32)

    start = np.asarray(inputs["crf_start"], np.float32)
    end = np.asarray(inputs["crf_end"], np.float32)
    trans = np.asarray(inputs["crf_trans"], np.float32)
    maskf = msk.astype(np.float32)

    em_sc = np.take_along_axis(emissions, tg[..., None], axis=2)[..., 0]
    tr_sc = trans[tg[:, :-1], tg[:, 1:]]
    last_idx = msk.sum(axis=1).astype(np.int64) - 1
    last_tag = tg[np.arange(B), last_idx]
    score = (
        start[tg[:, 0]]
        + em_sc[:, 0]
        + (maskf[:, 1:] * (tr_sc + em_sc[:, 1:])).sum(axis=1)
        + end[last_tag]
    )

    alpha = start[None, :] + emissions[:, 0]
    for t in range(1, S):
        new = _logsumexp(alpha[:, :, None] + trans[None] + emissions[:, t][:, None, :], axis=1)
        alpha = np.where(msk[:, t][:, None], new, alpha)
    logZ = _logsumexp(alpha + end[None, :], axis=1)
    return np.float32((logZ - score).sum())


def kernel(**inputs):
    msk = np.asarray(inputs["mask"]).astype(bool)
    use_device = (
        BF16 is not None
        and msk.all()
        and not os.environ.get("KERNEL_NO_DEVICE")
        and np.asarray(inputs["x"]).shape == (B, S)
    )
    if use_device:
        import signal

        old = None
        try:
            try:
                def _alarm(signum, frame):
                    raise TimeoutError("device path timed out")

                old = signal.signal(signal.SIGALRM, _alarm)
                signal.alarm(150)
            except ValueError:
                old = None  # not in main thread; run unguarded
            try:
                out = _run_device(inputs)
            finally:
                if old is not None:
                    signal.alarm(0)
                    signal.signal(signal.SIGALRM, old)
            if np.isfinite(out):
                return out
            print("device result not finite; falling back to host", file=sys.stderr)
        except Exception as e:  # noqa: BLE001
            print(f"device path failed ({e!r}); falling back to host", file=sys.stderr)
    return _run_host(inputs)

